# revision 4
# baseline (speedup 1.0000x reference)
"""DiceLoss partial-sum kernel for Trainium2 (8 NeuronCores, data-parallel).

Computes, for input/target of shape (32, 1, 1024, 1024) fp32:
    bin   = (input > 0.5) ? 1.0 : 0.0
    loss1 = 2 * sum(bin * target)
    loss2 = sum(bin) + sum(target)
and returns (loss1, loss2) as fp32 scalars (same structure as the reference).

Sharding: batch dim N=32 is split 4-per-core across 8 cores. Each core
views its 16 MiB input + 16 MiB target shard as a row-contiguous
[128, 32768] fp32 matrix (128 KiB DRAM rows) and streams column-chunk
loads through SBUF via HWDGE DMA. The problem is HBM-bound (16 HW DMA
engines sustain ~429 GB/s/core on 16 KB descriptors), so the design
keeps (a) the DMA queue always fed and (b) every engine's per-chunk
work under the ~9.8 us/4096-chunk DMA pace:
  loads:  per tensor: 8192, 8192, 8192, 4096, 2048, 1024, 512, 512 wide.
          The 8192-wide loads carry 32 KB descriptors (vs 16 KB) for
          better per-DMA-engine throughput. The first three cycle a
          2-slot 32 KB ring (load L waits consumers of load L-2); the
          4096 load reuses slot 1's first half; the 2048..512 taper
          loads go to a dedicated packed region with NO consumer
          coupling, so the final loads issue early and the DMA backend
          never stalls on compute (a BUFS=4 ring gated the taper loads
          on scalar's t-4 completion and cost ~3 us).
  vector: per 4096-chunk: STT (in>0.5)*tgt, accum -> loss1 col (4.4 us)
  scalar: per 4096-chunk: Copy(tgt) accum -> tgt col and Sign(1-2*in)
          accum -> sign col (~8.0 us incl. accum reads). The bin count
          is recovered on host as (count - S')/2, exact up to elements
          equal to 0.5 (~1e-8 relative).
  gpsimd: zeroes the stats tile once at start
Per-load semaphores (not per-slot) because HWDGE completions are not
ordered across dma_starts. Stats are per-chunk column triplets
[loss1, sign, tgt]; chunks 0..9 DMA out overlapped with the last
chunk's compute, then a tiny [128, 3] DMA ships the rest. The final
reduction over cores/partitions/chunks happens on the host in float64.
"""

from contextlib import ExitStack

import numpy as np

try:
    import concourse.bass  # noqa: F401
except ImportError:  # pragma: no cover - path fallback for bare containers
    import sys

    for _p in ("/opt/trn_rl_repo", "/root/.axon_site/_ro/trn_rl_repo"):
        if _p not in sys.path:
            sys.path.insert(0, _p)

import concourse.bacc as bacc
import concourse.mybir as mybir
from concourse.bass_utils import run_bass_kernel_spmd

N_CORES = 8
FULL_SHAPE = (32, 1, 1024, 1024)
FULL_ELEMS = 32 * 1024 * 1024
PER_CORE = FULL_ELEMS // N_CORES  # 4_194_304
P = 128
FREE = PER_CORE // P  # 32768 fp32 elements per partition per tensor
THRESH = 0.5

# (width, buffer, offset_in_buffer) per load; widths sum to FREE.
# buffer: 0/1 = 32 KB ring slots, 2 = 16 KB packed taper region.
LOADS = (
    (8192, 0, 0),
    (8192, 1, 0),
    (8192, 0, 0),
    (4096, 1, 0),
    (2048, 2, 0),
    (1024, 2, 2048),
    (512, 2, 3072),
    (512, 2, 3584),
)
assert sum(w for w, _, _ in LOADS) == FREE
# compute chunks: (width, load_idx, offset_in_load)
CHUNKS = (
    (4096, 0, 0), (4096, 0, 4096),
    (4096, 1, 0), (4096, 1, 4096),
    (4096, 2, 0), (4096, 2, 4096),
    (4096, 3, 0),
    (2048, 4, 0), (1024, 5, 0), (512, 6, 0), (512, 7, 0),
)
assert sum(w for w, _, _ in CHUNKS) == FREE
SLOT_W = 8192
TAPER_W = 4096

_CACHE: dict = {}


def _build(n_cores: int):
    f32 = mybir.dt.float32
    nch = len(CHUNKS)
    nc = bacc.Bacc(
        "TRN2", target_bir_lowering=False, debug=False, num_devices=n_cores
    )
    inp = nc.dram_tensor("input", [P * FREE], f32, kind="ExternalInput").ap()
    tgt = nc.dram_tensor("target", [P * FREE], f32, kind="ExternalInput").ap()
    stats = nc.dram_tensor("stats", [P, 3 * nch], f32, kind="ExternalOutput").ap()
    inpM = inp.rearrange("(p f) -> p f", p=P)
    tgtM = tgt.rearrange("(p f) -> p f", p=P)

    # buffers: two 8192-wide ring slots + one 4096-wide packed taper region
    ti = [
        nc.alloc_sbuf_tensor("ti0", [P, SLOT_W], f32).ap(),
        nc.alloc_sbuf_tensor("ti1", [P, SLOT_W], f32).ap(),
        nc.alloc_sbuf_tensor("ti2", [P, TAPER_W], f32).ap(),
    ]
    tt = [
        nc.alloc_sbuf_tensor("tt0", [P, SLOT_W], f32).ap(),
        nc.alloc_sbuf_tensor("tt1", [P, SLOT_W], f32).ap(),
        nc.alloc_sbuf_tensor("tt2", [P, TAPER_W], f32).ap(),
    ]
    # single write-only scratch per engine; each engine fully serializes
    # its own instructions via self-waits, so reuse is safe
    sd = nc.alloc_sbuf_tensor("sd", [P, 4096], f32).ap()
    sa = nc.alloc_sbuf_tensor("sa", [P, 4096], f32).ap()
    st = nc.alloc_sbuf_tensor("st", [P, 3 * nch], f32).ap()

    # DRAM column offset of each load
    lod_off = []
    off = 0
    for w, _, _ in LOADS:
        lod_off.append(off)
        off += w
    # consumer-instruction counts through chunk c
    V = [c + 1 for c in range(nch)]
    S = [2 * (c + 1) for c in range(nch)]
    # last chunk index consuming each load
    last_chunk_of_load = {}
    for c, (_, li, _) in enumerate(CHUNKS):
        last_chunk_of_load[li] = c

    with ExitStack() as ctx:
        load_sems = [
            ctx.enter_context(nc.semaphore(f"load_sem{i}"))
            for i in range(len(LOADS))
        ]
        vec_sem = ctx.enter_context(nc.semaphore("vec_sem"))
        sc_sem = ctx.enter_context(nc.semaphore("sc_sem"))
        gp_sem = ctx.enter_context(nc.semaphore("gp_sem"))
        out_sem = ctx.enter_context(nc.semaphore("out_sem"))
        block = ctx.enter_context(nc.Block())

        @block.gpsimd
        def _(gpsimd):
            gpsimd.memset(st[:], 0.0).then_inc(gp_sem, 1)

        @block.sync
        def _(sync):
            for li, (w, buf, boff) in enumerate(LOADS):
                if li >= 2 and buf < 2:
                    # ring slot reuse: consumers of the load two back in
                    # this slot must be done (load 2 -> load 0's chunks,
                    # load 3 -> load 1's chunks)
                    prev_last = last_chunk_of_load[li - 2]
                    sync.wait_ge(vec_sem, V[prev_last])
                    sync.wait_ge(sc_sem, S[prev_last])
                dst_i = ti[buf][:, boff : boff + w]
                dst_t = tt[buf][:, boff : boff + w]
                src_i = inpM[:, lod_off[li] : lod_off[li] + w]
                src_t = tgtM[:, lod_off[li] : lod_off[li] + w]
                sync.dma_start(out=dst_i, in_=src_i).then_inc(load_sems[li], 16)
                sync.dma_start(out=dst_t, in_=src_t).then_inc(load_sems[li], 16)
            # stats for chunks 0..nch-2 ship overlapped with the last
            # chunk's compute; the last triplet ships at the very end
            sync.wait_ge(vec_sem, V[-2])
            sync.wait_ge(sc_sem, S[-2])
            sync.wait_ge(gp_sem, 1)
            head = 3 * (nch - 1)
            sync.dma_start(out=stats[:, :head], in_=st[:, :head]).then_inc(
                out_sem, 16
            )
            sync.wait_ge(vec_sem, V[-1])
            sync.wait_ge(sc_sem, S[-1])
            sync.dma_start(out=stats[:, head:], in_=st[:, head:]).then_inc(
                out_sem, 16
            )
            sync.wait_ge(out_sem, 32)

        @block.vector
        def _(vector):
            vector.wait_ge(gp_sem, 1)
            vi = 0
            for c, (w, li, coff) in enumerate(CHUNKS):
                buf, boff = LOADS[li][1], LOADS[li][2] + coff
                vector.wait_ge(load_sems[li], 32)
                if vi >= 1:
                    # single shared scratch: full serialization on retirement
                    vector.wait_ge(vec_sem, vi)
                vector.scalar_tensor_tensor(
                    out=sd[:, :w],
                    in0=ti[buf][:, boff : boff + w],
                    scalar=THRESH,
                    in1=tt[buf][:, boff : boff + w],
                    op0=mybir.AluOpType.is_gt,
                    op1=mybir.AluOpType.mult,
                    accum_out=st[:, 3 * c : 3 * c + 1],
                ).then_inc(vec_sem, 1)
                vi += 1

        @block.scalar
        def _(scalar):
            scalar.wait_ge(gp_sem, 1)
            si = 0
            for c, (w, li, coff) in enumerate(CHUNKS):
                buf, boff = LOADS[li][1], LOADS[li][2] + coff
                scalar.wait_ge(load_sems[li], 32)
                if si >= 1:
                    scalar.wait_ge(sc_sem, si)
                scalar.activation(
                    out=sa[:, :w],
                    in_=tt[buf][:, boff : boff + w],
                    func=mybir.ActivationFunctionType.Copy,
                    accum_out=st[:, 3 * c + 2 : 3 * c + 3],
                ).then_inc(sc_sem, 1)
                si += 1
                scalar.wait_ge(sc_sem, si)
                # Sign(1 - 2x) = -Sign(x - 0.5); bias=1.0 has a pre-registered
                # const AP; host converts the sum to a >0.5 count
                scalar.activation(
                    out=sa[:, :w],
                    in_=ti[buf][:, boff : boff + w],
                    func=mybir.ActivationFunctionType.Sign,
                    bias=1.0,
                    scale=-2.0,
                    accum_out=st[:, 3 * c + 1 : 3 * c + 2],
                ).then_inc(sc_sem, 1)
                si += 1

    nc.compile()
    return nc


def _get_nc():
    key = N_CORES
    if key not in _CACHE:
        _CACHE[key] = _build(key)
    return _CACHE[key]


def kernel(input: np.ndarray, target: np.ndarray, **run_kwargs):
    inp = np.asarray(input, dtype=np.float32).reshape(N_CORES, PER_CORE)
    tgt = np.asarray(target, dtype=np.float32).reshape(N_CORES, PER_CORE)

    nc = _get_nc()
    in_maps = [
        {"input": np.ascontiguousarray(inp[c]), "target": np.ascontiguousarray(tgt[c])}
        for c in range(N_CORES)
    ]
    res = run_bass_kernel_spmd(nc, in_maps, core_ids=list(range(N_CORES)), **run_kwargs)

    nch = len(CHUNKS)
    inter = 0.0
    sign_sum = 0.0
    tgt_sum = 0.0
    for c in range(N_CORES):
        stats = res.results[c]["stats"].astype(np.float64).reshape(P, nch, 3)
        inter += stats[:, :, 0].sum()
        sign_sum += stats[:, :, 1].sum()
        tgt_sum += stats[:, :, 2].sum()
    # bin count from sign sums: S' = #lt - #gt -> count(>thr) = (n - S')/2
    loss2 = tgt_sum + (FULL_ELEMS - sign_sum) / 2.0

    loss1 = np.float32(2.0 * inter)
    loss2 = np.float32(loss2)
    out = (loss1, loss2)
    if run_kwargs.get("trace"):
        return out, res
    return out


# revision 5
# speedup vs baseline: 1.0631x; 1.0631x over previous
"""DiceLoss partial-sum kernel for Trainium2 (8 NeuronCores, data-parallel).

Computes, for input/target of shape (32, 1, 1024, 1024) fp32:
    bin   = (input > 0.5) ? 1.0 : 0.0
    loss1 = 2 * sum(bin * target)
    loss2 = sum(bin) + sum(target)
and returns (loss1, loss2) as fp32 scalars (same structure as the reference).

Sharding: batch dim N=32 is split 4-per-core across 8 cores. Each core
streams its 16 MiB input + 16 MiB target shard through SBUF as [128, F]
fp32 tiles via HWDGE DMA on the sync queue. The problem is HBM-bound
(~429 GB/s/core sustained over 16 HW DMA engines = ~9.8 us per 4096-wide
tile pair), so the design keeps (a) the DMA descriptor queue always fed
and (b) every engine's per-tile work well under the DMA pace:
  loads:  tiles 0-6 are 4096 wide through a 4-slot SBUF ring; the slot
          reuse waits (tile t on consumers of t-4) are satisfied tens of
          us before the DMA backend reaches them, so the queue never
          starves. The 2048/1024/512/512 taper tiles go to dedicated
          packed buffers with NO consumer coupling - their dma_starts
          follow tile 6's immediately, so the backend finishes with the
          smallest tiles and the compute tail is ~2 us. (Variants that
          gated taper loads on ring reuse, or used 2-slot rings of
          8192-wide loads, stalled the backend mid-stream: completions
          of a dma_start pair arrive ~2.3 us/MiB after its descriptors
          start, so big loads also delay the first consumable tile.
          8192-wide loads' 32 KB descriptors gained only ~1% bandwidth.)
  vector: every tile: STT (in>0.5)*tgt, accum -> loss1 col; on "dve"
          tiles also STT (in>0.5)+tgt, accum -> loss2 col (exact).
          Outputs go to PSUM (write-only sink, frees SBUF).
  scalar: on "act" tiles: Copy(tgt) accum -> tgt col and Sign(1-2*in)
          accum -> sign col; bin count recovered on host as
          (count - S')/2, exact up to elements equal to 0.5 (~1e-8 rel).
  gpsimd: zeroes the stats tile once at start.
The dve/act alternation keeps vector at ~68% and scalar at ~41% of the
DMA pace, so the pipeline tolerates the ~20% engine-clock p-state
throttling this part sometimes shows without the ring backing up.
Per-load semaphores on the taper (HWDGE completions are NOT ordered
across dma_starts). Stats are per-tile column triplets; tiles 0..nt-2
DMA out overlapped with the last tile's compute, then a tiny [128, 3]
DMA ships the rest. Final reduction happens on the host in float64.
"""

from contextlib import ExitStack

import numpy as np

try:
    import concourse.bass  # noqa: F401
except ImportError:  # pragma: no cover - path fallback for bare containers
    import sys

    for _p in ("/opt/trn_rl_repo", "/root/.axon_site/_ro/trn_rl_repo"):
        if _p not in sys.path:
            sys.path.insert(0, _p)

import concourse.bacc as bacc
import concourse.mybir as mybir
from concourse.bass_utils import run_bass_kernel_spmd

N_CORES = 8
FULL_SHAPE = (32, 1, 1024, 1024)
FULL_ELEMS = 32 * 1024 * 1024
PER_CORE = FULL_ELEMS // N_CORES  # 4_194_304
P = 128
FREE = PER_CORE // P  # 32768 fp32 elements per partition per tensor
THRESH = 0.5
BUFS = 4  # ring depth for the 4096-wide tiles

# (width, mode); widths sum to FREE. Tiles 0-6 ring; 7-10 are dedicated.
TILES = (
    (4096, "act"), (4096, "dve"), (4096, "act"), (4096, "dve"),
    (4096, "act"), (4096, "dve"), (4096, "act"),
    (2048, "dve"), (1024, "act"), (512, "dve"), (512, "act"),
)
assert sum(w for w, _ in TILES) == FREE
N_RING = 7  # tiles 0-6 go through the ring
# packed offsets of taper tiles 7-10 inside the 4096-wide taper buffer
TAPER_OFF = (0, 2048, 3072, 3584)

_CACHE: dict = {}


def _build(n_cores: int):
    f32 = mybir.dt.float32
    nt = len(TILES)
    nc = bacc.Bacc(
        "TRN2", target_bir_lowering=False, debug=False, num_devices=n_cores
    )
    inp = nc.dram_tensor("input", [P * FREE], f32, kind="ExternalInput").ap()
    tgt = nc.dram_tensor("target", [P * FREE], f32, kind="ExternalInput").ap()
    stats = nc.dram_tensor("stats", [P, 3 * nt], f32, kind="ExternalOutput").ap()

    ti_ring = nc.alloc_sbuf_tensor("ti_ring", [P, BUFS * 4096], f32).ap()
    tt_ring = nc.alloc_sbuf_tensor("tt_ring", [P, BUFS * 4096], f32).ap()
    ti_tap = nc.alloc_sbuf_tensor("ti_tap", [P, 4096], f32).ap()
    tt_tap = nc.alloc_sbuf_tensor("tt_tap", [P, 4096], f32).ap()
    # vector's write-only STT sink lives in PSUM (exactly 4096 fp32/part);
    # self-waits serialize retirement so single-buffer reuse is safe
    sd = nc.alloc_psum_tensor("sd", [P, 4096], f32).ap()
    # scalar alternates two SBUF sinks (deep-pipeline WAW)
    sa = [nc.alloc_sbuf_tensor(f"sa{i}", [P, 4096], f32).ap() for i in range(2)]
    st = nc.alloc_sbuf_tensor("st", [P, 3 * nt], f32).ap()

    offs = []
    off = 0
    for w, _ in TILES:
        offs.append(off)
        off += P * w

    # cumulative consumer-instruction counts through tile t
    V, S = [], []
    v = s = 0
    for w, mode in TILES:
        v += 2 if mode == "dve" else 1
        s += 0 if mode == "dve" else 2
        V.append(v)
        S.append(s)

    def src(t, ap):
        w = TILES[t][0]
        return ap[offs[t] : offs[t] + P * w].rearrange("(p f) -> p f", p=P)

    def dst(t, ring_ap, tap_ap):
        w = TILES[t][0]
        if t < N_RING:
            s_ = (t % BUFS) * 4096
            return ring_ap[:, s_ : s_ + w]
        s_ = TAPER_OFF[t - N_RING]
        return tap_ap[:, s_ : s_ + w]

    with ExitStack() as ctx:
        slot_sems = [
            ctx.enter_context(nc.semaphore(f"slot_sem{i}")) for i in range(BUFS)
        ]
        tap_sems = [
            ctx.enter_context(nc.semaphore(f"tap_sem{i}"))
            for i in range(nt - N_RING)
        ]
        vec_sem = ctx.enter_context(nc.semaphore("vec_sem"))
        sc_sem = ctx.enter_context(nc.semaphore("sc_sem"))
        gp_sem = ctx.enter_context(nc.semaphore("gp_sem"))
        out_sem = ctx.enter_context(nc.semaphore("out_sem"))
        block = ctx.enter_context(nc.Block())

        def tile_sem_wait(eng, t):
            if t < N_RING:
                eng.wait_ge(slot_sems[t % BUFS], 32 * (t // BUFS + 1))
            else:
                eng.wait_ge(tap_sems[t - N_RING], 32)

        @block.gpsimd
        def _(gpsimd):
            gpsimd.memset(st[:], 0.0).then_inc(gp_sem, 1)

        @block.sync
        def _(sync):
            for t, (w, mode) in enumerate(TILES):
                if BUFS <= t < N_RING:
                    # ring slot reuse: consumers of tile t-BUFS must be done
                    sync.wait_ge(vec_sem, V[t - BUFS])
                    if S[t - BUFS] > 0:
                        sync.wait_ge(sc_sem, S[t - BUFS])
                sem = slot_sems[t % BUFS] if t < N_RING else tap_sems[t - N_RING]
                sync.dma_start(out=dst(t, ti_ring, ti_tap), in_=src(t, inp)).then_inc(
                    sem, 16
                )
                sync.dma_start(out=dst(t, tt_ring, tt_tap), in_=src(t, tgt)).then_inc(
                    sem, 16
                )
            # sem update on an accum instruction fires at full instruction
            # retirement (incl. the accumulator write-back), so the stats DMAs
            # can depend on the compute sems directly - no fence instructions.
            sync.wait_ge(vec_sem, V[-2])
            sync.wait_ge(sc_sem, S[-2])
            sync.wait_ge(gp_sem, 1)
            head = 3 * (nt - 1)
            sync.dma_start(out=stats[:, :head], in_=st[:, :head]).then_inc(
                out_sem, 16
            )
            sync.wait_ge(vec_sem, V[-1])
            sync.wait_ge(sc_sem, S[-1])
            sync.dma_start(out=stats[:, head:], in_=st[:, head:]).then_inc(
                out_sem, 16
            )
            sync.wait_ge(out_sem, 32)

        @block.vector
        def _(vector):
            vector.wait_ge(gp_sem, 1)
            vi = 0
            for t, (w, mode) in enumerate(TILES):
                in_i = dst(t, ti_ring, ti_tap)
                in_t = dst(t, tt_ring, tt_tap)
                tile_sem_wait(vector, t)
                ops = [(mybir.AluOpType.mult, 0)]
                if mode == "dve":
                    ops.append((mybir.AluOpType.add, 2))
                for op1, col in ops:
                    if vi >= 1:
                        # single PSUM sink: serialize on retirement
                        vector.wait_ge(vec_sem, vi)
                    vector.scalar_tensor_tensor(
                        out=sd[:, :w],
                        in0=in_i,
                        scalar=THRESH,
                        in1=in_t,
                        op0=mybir.AluOpType.is_gt,
                        op1=op1,
                        accum_out=st[:, 3 * t + col : 3 * t + col + 1],
                    ).then_inc(vec_sem, 1)
                    vi += 1

        @block.scalar
        def _(scalar):
            scalar.wait_ge(gp_sem, 1)
            si = 0
            for t, (w, mode) in enumerate(TILES):
                if mode == "dve":
                    continue
                in_i = dst(t, ti_ring, ti_tap)
                in_t = dst(t, tt_ring, tt_tap)
                tile_sem_wait(scalar, t)
                if si >= 2:
                    scalar.wait_ge(sc_sem, si - 1)
                scalar.activation(
                    out=sa[si % 2][:, :w],
                    in_=in_t,
                    func=mybir.ActivationFunctionType.Copy,
                    accum_out=st[:, 3 * t + 2 : 3 * t + 3],
                ).then_inc(sc_sem, 1)
                si += 1
                if si >= 2:
                    scalar.wait_ge(sc_sem, si - 1)
                # Sign(1 - 2x) = -Sign(x - 0.5); bias=1.0 has a pre-registered
                # const AP; host converts the sum to a >0.5 count
                scalar.activation(
                    out=sa[si % 2][:, :w],
                    in_=in_i,
                    func=mybir.ActivationFunctionType.Sign,
                    bias=1.0,
                    scale=-2.0,
                    accum_out=st[:, 3 * t + 1 : 3 * t + 2],
                ).then_inc(sc_sem, 1)
                si += 1

    nc.compile()
    return nc


def _get_nc():
    key = N_CORES
    if key not in _CACHE:
        _CACHE[key] = _build(key)
    return _CACHE[key]


def kernel(input: np.ndarray, target: np.ndarray, **run_kwargs):
    inp = np.asarray(input, dtype=np.float32).reshape(N_CORES, PER_CORE)
    tgt = np.asarray(target, dtype=np.float32).reshape(N_CORES, PER_CORE)

    nc = _get_nc()
    in_maps = [
        {"input": np.ascontiguousarray(inp[c]), "target": np.ascontiguousarray(tgt[c])}
        for c in range(N_CORES)
    ]
    res = run_bass_kernel_spmd(nc, in_maps, core_ids=list(range(N_CORES)), **run_kwargs)

    nt = len(TILES)
    act = [t for t, (_, m) in enumerate(TILES) if m == "act"]
    dve = [t for t, (_, m) in enumerate(TILES) if m == "dve"]
    inter = 0.0
    loss2 = 0.0
    sign_sum = 0.0
    for c in range(N_CORES):
        stats = res.results[c]["stats"].astype(np.float64).reshape(P, nt, 3)
        inter += stats[:, :, 0].sum()
        # dve tiles: col 2 holds direct (bin + tgt) partials
        loss2 += stats[:, dve, 2].sum()
        # act tiles: col 2 holds tgt sums, col 1 holds sign sums
        loss2 += stats[:, act, 2].sum()
        sign_sum += stats[:, act, 1].sum()
    n_act_elems = N_CORES * P * sum(TILES[t][0] for t in act)
    # bin count from sign sums: S' = #lt - #gt -> count(>thr) = (n - S')/2
    loss2 += (n_act_elems - sign_sum) / 2.0

    loss1 = np.float32(2.0 * inter)
    loss2 = np.float32(loss2)
    out = (loss1, loss2)
    if run_kwargs.get("trace"):
        return out, res
    return out


# revision 6
# speedup vs baseline: 1.2286x; 1.1557x over previous
"""DiceLoss partial-sum kernel for Trainium2 (8 NeuronCores, data-parallel).

Computes, for input/target of shape (32, 1, 1024, 1024) fp32:
    bin   = (input > 0.5) ? 1.0 : 0.0
    loss1 = 2 * sum(bin * target)
    loss2 = sum(bin) + sum(target)
and returns (loss1, loss2) as fp32 scalars (same structure as the reference).

Sharding: batch dim N=32 is split 4-per-core across 8 cores. Each core
streams its 16 MiB input + 16 MiB target shard through SBUF as [128, F]
fp32 tiles via HWDGE DMA on the sync queue. The problem is HBM-bound
(~429 GB/s/core sustained over 16 HW DMA engines = ~9.8 us per 4096-wide
tile pair; 8192-wide loads' 32 KB descriptors gained only ~1%), so the
design keeps the DMA descriptor queue fed but NOT overflowed, and every
engine's per-tile work under the DMA pace:
  loads:  7x4096 + 2048 + 2x1024 wide tiles through a 4-slot SBUF ring.
          Slot reuse (tile t waits consumers of t-4) paces descriptor
          submission: with the dve/act compute split the waits fire
          ~10 us or more before the backend drains the queue, so DMA
          never starves - but they DO hold the tail tiles' descriptors
          back enough that the HWDGE descriptor ring never fills. (A
          variant issuing the taper loads with no consumer coupling
          overflowed the ring: the dma_starts themselves blocked
          4-10 us each in descriptor generation, and the tail data
          slipped ~15 us.)
  vector: every tile: STT (in>0.5)*tgt, accum -> loss1 col; on "dve"
          tiles also STT (in>0.5)+tgt, accum -> loss2 col (exact).
          STT sink is a single PSUM buffer (write-only; self-waits
          serialize retirement, and PSUM use frees SBUF).
  scalar: on "act" tiles: Copy(tgt) accum -> tgt col and Sign(1-2*in)
          accum -> sign col; bin count recovered on host as
          (count - S')/2, exact up to elements equal to 0.5 (~1e-8 rel).
  gpsimd: zeroes the stats tile once at start.
The dve/act alternation keeps vector at ~68% and scalar at ~41% of the
DMA pace on average, so the pipeline tolerates the ~20% engine-clock
p-state throttling this part sometimes shows. The tile taper (last
tiles 2048/1024/1024) keeps the post-last-byte compute tail to ~1.5 us.
Stats are per-tile column triplets; tiles 0..nt-2 DMA out overlapped
with the last tile's compute, then a tiny [128, 3] DMA ships the rest.
Final reduction over cores/partitions/tiles happens on the host in
float64.
"""

from contextlib import ExitStack

import numpy as np

try:
    import concourse.bass  # noqa: F401
except ImportError:  # pragma: no cover - path fallback for bare containers
    import sys

    for _p in ("/opt/trn_rl_repo", "/root/.axon_site/_ro/trn_rl_repo"):
        if _p not in sys.path:
            sys.path.insert(0, _p)

import concourse.bacc as bacc
import concourse.mybir as mybir
from concourse.bass_utils import run_bass_kernel_spmd

N_CORES = 8
FULL_SHAPE = (32, 1, 1024, 1024)
FULL_ELEMS = 32 * 1024 * 1024
PER_CORE = FULL_ELEMS // N_CORES  # 4_194_304
P = 128
FREE = PER_CORE // P  # 32768 fp32 elements per partition per tensor
THRESH = 0.5
BUFS = 4  # SBUF ring depth per tensor (4 x 16 KiB rows per partition)

# (width, mode); widths sum to FREE. "dve" tiles compute loss2 on vector
# (one extra STT), "act" tiles on scalar (Copy + Sign). The last tile is
# dve so the final tail is a pair of short vector ops.
TILES = (
    (4096, "act"), (4096, "dve"), (4096, "act"), (4096, "dve"),
    (4096, "act"), (4096, "dve"), (4096, "act"),
    (2048, "dve"), (1024, "act"), (1024, "dve"),
)
assert sum(w for w, _ in TILES) == FREE

_CACHE: dict = {}


def _build(n_cores: int):
    f32 = mybir.dt.float32
    nt = len(TILES)
    nc = bacc.Bacc(
        "TRN2", target_bir_lowering=False, debug=False, num_devices=n_cores
    )
    inp = nc.dram_tensor("input", [P * FREE], f32, kind="ExternalInput").ap()
    tgt = nc.dram_tensor("target", [P * FREE], f32, kind="ExternalInput").ap()
    stats = nc.dram_tensor("stats", [P, 3 * nt], f32, kind="ExternalOutput").ap()

    ti_ring = nc.alloc_sbuf_tensor("ti_ring", [P, BUFS * 4096], f32).ap()
    tt_ring = nc.alloc_sbuf_tensor("tt_ring", [P, BUFS * 4096], f32).ap()
    # vector's write-only STT sink lives in PSUM (exactly 4096 fp32/part);
    # self-waits serialize retirement so single-buffer reuse is safe
    sd = nc.alloc_psum_tensor("sd", [P, 4096], f32).ap()
    # scalar alternates two SBUF sinks (deep-pipeline WAW)
    sa = [nc.alloc_sbuf_tensor(f"sa{i}", [P, 4096], f32).ap() for i in range(2)]
    st = nc.alloc_sbuf_tensor("st", [P, 3 * nt], f32).ap()

    offs = []
    off = 0
    for w, _ in TILES:
        offs.append(off)
        off += P * w

    # cumulative consumer-instruction counts through tile t
    V, S = [], []
    v = s = 0
    for w, mode in TILES:
        v += 2 if mode == "dve" else 1
        s += 0 if mode == "dve" else 2
        V.append(v)
        S.append(s)

    def src(t, ap):
        w = TILES[t][0]
        return ap[offs[t] : offs[t] + P * w].rearrange("(p f) -> p f", p=P)

    def ring(t, ring_ap):
        w = TILES[t][0]
        s_ = (t % BUFS) * 4096
        return ring_ap[:, s_ : s_ + w]

    with ExitStack() as ctx:
        slot_sems = [
            ctx.enter_context(nc.semaphore(f"slot_sem{i}")) for i in range(BUFS)
        ]
        vec_sem = ctx.enter_context(nc.semaphore("vec_sem"))
        sc_sem = ctx.enter_context(nc.semaphore("sc_sem"))
        gp_sem = ctx.enter_context(nc.semaphore("gp_sem"))
        out_sem = ctx.enter_context(nc.semaphore("out_sem"))
        block = ctx.enter_context(nc.Block())

        @block.gpsimd
        def _(gpsimd):
            gpsimd.memset(st[:], 0.0).then_inc(gp_sem, 1)

        @block.sync
        def _(sync):
            for t, (w, mode) in enumerate(TILES):
                if t >= BUFS:
                    # ring slot reuse: consumers of tile t-BUFS must be done
                    sync.wait_ge(vec_sem, V[t - BUFS])
                    if S[t - BUFS] > 0:
                        sync.wait_ge(sc_sem, S[t - BUFS])
                sem = slot_sems[t % BUFS]
                sync.dma_start(out=ring(t, ti_ring), in_=src(t, inp)).then_inc(
                    sem, 16
                )
                sync.dma_start(out=ring(t, tt_ring), in_=src(t, tgt)).then_inc(
                    sem, 16
                )
            # sem update on an accum instruction fires at full instruction
            # retirement (incl. the accumulator write-back), so the stats DMAs
            # can depend on the compute sems directly - no fence instructions.
            sync.wait_ge(vec_sem, V[-2])
            sync.wait_ge(sc_sem, S[-2])
            sync.wait_ge(gp_sem, 1)
            head = 3 * (nt - 1)
            sync.dma_start(out=stats[:, :head], in_=st[:, :head]).then_inc(
                out_sem, 16
            )
            sync.wait_ge(vec_sem, V[-1])
            sync.wait_ge(sc_sem, S[-1])
            sync.dma_start(out=stats[:, head:], in_=st[:, head:]).then_inc(
                out_sem, 16
            )
            sync.wait_ge(out_sem, 32)

        @block.vector
        def _(vector):
            vector.wait_ge(gp_sem, 1)
            vi = 0
            for t, (w, mode) in enumerate(TILES):
                in_i = ring(t, ti_ring)
                in_t = ring(t, tt_ring)
                vector.wait_ge(slot_sems[t % BUFS], 32 * (t // BUFS + 1))
                ops = [(mybir.AluOpType.mult, 0)]
                if mode == "dve":
                    ops.append((mybir.AluOpType.add, 2))
                for op1, col in ops:
                    if vi >= 1:
                        # single PSUM sink: serialize on retirement
                        vector.wait_ge(vec_sem, vi)
                    vector.scalar_tensor_tensor(
                        out=sd[:, :w],
                        in0=in_i,
                        scalar=THRESH,
                        in1=in_t,
                        op0=mybir.AluOpType.is_gt,
                        op1=op1,
                        accum_out=st[:, 3 * t + col : 3 * t + col + 1],
                    ).then_inc(vec_sem, 1)
                    vi += 1

        @block.scalar
        def _(scalar):
            scalar.wait_ge(gp_sem, 1)
            si = 0
            for t, (w, mode) in enumerate(TILES):
                if mode == "dve":
                    continue
                in_i = ring(t, ti_ring)
                in_t = ring(t, tt_ring)
                scalar.wait_ge(slot_sems[t % BUFS], 32 * (t // BUFS + 1))
                if si >= 2:
                    scalar.wait_ge(sc_sem, si - 1)
                scalar.activation(
                    out=sa[si % 2][:, :w],
                    in_=in_t,
                    func=mybir.ActivationFunctionType.Copy,
                    accum_out=st[:, 3 * t + 2 : 3 * t + 3],
                ).then_inc(sc_sem, 1)
                si += 1
                if si >= 2:
                    scalar.wait_ge(sc_sem, si - 1)
                # Sign(1 - 2x) = -Sign(x - 0.5); bias=1.0 has a pre-registered
                # const AP; host converts the sum to a >0.5 count
                scalar.activation(
                    out=sa[si % 2][:, :w],
                    in_=in_i,
                    func=mybir.ActivationFunctionType.Sign,
                    bias=1.0,
                    scale=-2.0,
                    accum_out=st[:, 3 * t + 1 : 3 * t + 2],
                ).then_inc(sc_sem, 1)
                si += 1

    nc.compile()
    return nc


def _get_nc():
    key = N_CORES
    if key not in _CACHE:
        _CACHE[key] = _build(key)
    return _CACHE[key]


def kernel(input: np.ndarray, target: np.ndarray, **run_kwargs):
    inp = np.asarray(input, dtype=np.float32).reshape(N_CORES, PER_CORE)
    tgt = np.asarray(target, dtype=np.float32).reshape(N_CORES, PER_CORE)

    nc = _get_nc()
    in_maps = [
        {"input": np.ascontiguousarray(inp[c]), "target": np.ascontiguousarray(tgt[c])}
        for c in range(N_CORES)
    ]
    res = run_bass_kernel_spmd(nc, in_maps, core_ids=list(range(N_CORES)), **run_kwargs)

    nt = len(TILES)
    act = [t for t, (_, m) in enumerate(TILES) if m == "act"]
    dve = [t for t, (_, m) in enumerate(TILES) if m == "dve"]
    inter = 0.0
    loss2 = 0.0
    sign_sum = 0.0
    for c in range(N_CORES):
        stats = res.results[c]["stats"].astype(np.float64).reshape(P, nt, 3)
        inter += stats[:, :, 0].sum()
        # dve tiles: col 2 holds direct (bin + tgt) partials
        loss2 += stats[:, dve, 2].sum()
        # act tiles: col 2 holds tgt sums, col 1 holds sign sums
        loss2 += stats[:, act, 2].sum()
        sign_sum += stats[:, act, 1].sum()
    n_act_elems = N_CORES * P * sum(TILES[t][0] for t in act)
    # bin count from sign sums: S' = #lt - #gt -> count(>thr) = (n - S')/2
    loss2 += (n_act_elems - sign_sum) / 2.0

    loss1 = np.float32(2.0 * inter)
    loss2 = np.float32(loss2)
    out = (loss1, loss2)
    if run_kwargs.get("trace"):
        return out, res
    return out
